# revision 17
# baseline (speedup 1.0000x reference)
"""DLASSO unfolded GNN message-passing kernel for 8 Trainium2 NeuronCores.

Sharding: P (graph partitions / per-partition Gram matrices) split 32/core.
  - AtA built on device (fp32r matmuls), stored bf16 in SBUF (16 MB/core),
    streamed through the PE as the moving operand each step with the per-
    partition y vectors (8 batches) as an 8-column stationary operand.
    Four partitions share each PSUM bank via col-tiling (tile_position).
  - State tensors live in packed [16p*8b, 512] fp32 SBUF tiles; PSUM
    results are extracted with wide ACT copies into a 32-row-strided padded
    tile and re-laned with small partition-sliced SBUF->SBUF DMAs.
  - The graph scatter (delta = 2*deg*y - Adj@y) is batch-local: each step
    reshards y P->B (AllToAll #1), computes delta/U and C = U*deg+delta*rho
    on the batch-sharded side (core c owns batch c, Adj matmul in fp32r),
    and ships C back B->P (AllToAll #2), overlapping the next matvec.
"""

import sys

sys.path.insert(0, "/opt/trn_rl_repo")

import ml_dtypes
import numpy as np

import concourse.bass as bass
import concourse.mybir as mybir
import concourse.tile as tile
from concourse.bass_utils import run_bass_kernel_spmd

B, P, M, N, K = 8, 256, 256, 512, 15
NCORES = 8
PL = P // NCORES          # 32 partitions per core
NT = 2                    # packed state tiles per tensor
PPT = 16                  # p's per packed tile
NG = PL // 4              # 8 psum groups of 4 p's
LCH = 4                   # 128-row l-chunks in the contraction dim
F32 = mybir.dt.float32
F32R = mybir.dt.float32r
BF16 = mybir.dt.bfloat16
TT = mybir.AluOpType
AFT = mybir.ActivationFunctionType


def _fix_multiwait(nc):
    """walrus here caps sem-waits per instruction at 1 (2 for EventSemaphore);
    Tile's kernel-tail drain can carry more - hoist extras onto NOPs."""
    cnt = 0
    for f in nc.m.functions:
        for bb in f.blocks:
            out, changed = [], False
            for ins in bb.instructions:
                si = ins.sync_info
                cap = 2 if isinstance(ins, mybir.InstEventSemaphore) else 1
                if si is not None and len(si.on_wait) > cap:
                    waits = list(si.on_wait)
                    for w in waits[:-cap]:
                        cnt += 1
                        out.append(mybir.InstNoOp(
                            name=f"waitfix-{cnt}", engine=ins.engine,
                            bass_nofuse=True,
                            sync_info=mybir.SyncInfo(on_wait=[w], on_update=[])))
                    ins.sync_info = mybir.SyncInfo(
                        on_wait=waits[-cap:], on_update=list(si.on_update))
                    changed = True
                out.append(ins)
            if changed:
                bb.instructions = out
    return cnt


def _build_nc(gmax, ymax):
    nc = bass.Bass()

    A_in = nc.dram_tensor("A_in", [PL, 2, 128, N], F32R, kind="ExternalInput")
    bT_in = nc.dram_tensor("bT_in", [NG, 2, 128, 32], F32R, kind="ExternalInput")
    y0_in = nc.dram_tensor("y0_in", [NT, 128, N], F32, kind="ExternalInput")
    c0_in = nc.dram_tensor("c0_in", [NT, 128, N], F32, kind="ExternalInput")
    ub0_in = nc.dram_tensor("ub0_in", [2, 128, N], F32, kind="ExternalInput")
    adj_in = nc.dram_tensor("adj_in", [2, 128, P], F32R, kind="ExternalInput")
    # per-partition scalar columns: [128, x]
    scalp_in = nc.dram_tensor("scalp_in", [128, 2 * K * NT], F32, kind="ExternalInput")
    scalb_in = nc.dram_tensor("scalb_in", [128, 2 * (K - 1) * 2 + 4], F32,
                              kind="ExternalInput")
    ident_in = nc.dram_tensor("ident_in", [128, 128], F32, kind="ExternalInput")
    Y_out = nc.dram_tensor("Y_out", [K, B, PL, N], F32, kind="ExternalOutput")

    rg = [list(range(NCORES))]

    with tile.TileContext(nc) as tc:
        with (
            tc.tile_pool(name="const", bufs=1) as constp,
            tc.tile_pool(name="ata", bufs=1) as atap,
            tc.tile_pool(name="state", bufs=1) as statep,
            tc.tile_pool(name="dram", bufs=1, space="DRAM") as dram,
        ):
            atabf = [atap.tile([128, PL * N], BF16, name=f"atabf{c}") for c in range(LCH)]
            y_t = [statep.tile([128, N], F32, name=f"y{t}") for t in range(NT)]
            w_t = [statep.tile([128, N], F32, name=f"w{t}") for t in range(NT)]
            atb_t = [statep.tile([128, N], F32, name=f"atb{t}") for t in range(NT)]
            c_t = [statep.tile([128, N], F32, name=f"c{t}") for t in range(NT)]
            ub_t = [statep.tile([128, N], F32, name=f"ub{t}") for t in range(NT)]
            yf_t = [statep.tile([128, N], F32, name=f"yf{t}") for t in range(2)]
            yt_c = [statep.tile([128, PL * B], BF16, name=f"yt{c}") for c in range(LCH)]
            adj_t = [constp.tile([128, P], F32R, name=f"adj{q}") for q in range(2)]
            scalp = constp.tile([128, 2 * K * NT], F32, name="scalp")
            scalb = constp.tile([128, 2 * (K - 1) * 2 + 4], F32, name="scalb")
            ident = constp.tile([128, 128], F32, name="ident")

            def alpha(k, t):  # [128,1] per-partition alpha_k rows (p,b)
                return scalp[:, 2 * (k * NT + t):2 * (k * NT + t) + 1]

            def taucol(k, t):
                return scalp[:, 2 * (k * NT + t) + 1:2 * (k * NT + t) + 2]

            def rhocol(kb, t):  # B-side rows = p; rho_{kb+1}
                return scalb[:, 2 * (kb * 2 + t):2 * (kb * 2 + t) + 1]

            def etacol(kb, t):
                return scalb[:, 2 * (kb * 2 + t) + 1:2 * (kb * 2 + t) + 2]

            def degb2(t):
                return scalb[:, 2 * (K - 1) * 2 + t:2 * (K - 1) * 2 + t + 1]

            def degb(t):
                return scalb[:, 2 * (K - 1) * 2 + 2 + t:2 * (K - 1) * 2 + 3 + t]

            for q in range(2):
                nc.sync.dma_start(out=adj_t[q][:], in_=adj_in[q])
            nc.sync.dma_start(out=scalp[:], in_=scalp_in[:])
            nc.sync.dma_start(out=scalb[:], in_=scalb_in[:])
            nc.sync.dma_start(out=ident[:], in_=ident_in[:])
            for t in range(NT):
                nc.sync.dma_start(out=y_t[t][:], in_=y0_in[t])
                nc.sync.dma_start(out=c_t[t][:], in_=c0_in[t])
                nc.sync.dma_start(out=ub_t[t][:], in_=ub0_in[t])

            a2a1_in = dram.tile([B, PL, N], F32, name="a2a1_in")
            a2a1_out = dram.tile([P, N], F32, name="a2a1_out")
            a2a2_in = dram.tile([P, N], F32, name="a2a2_in")
            a2a2_out = dram.tile([B, PL, N], F32, name="a2a2_out")

            # ---- setup: AtA (bf16) and Atb on device ------------------------
            with (
                tc.tile_pool(name="aload", bufs=5) as aload,
                tc.tile_pool(name="bpsum", bufs=4, space="PSUM") as bpsum,
                tc.tile_pool(name="bpad", bufs=2) as bpadp,
            ):
                ats = []
                for p in range(PL):
                    at = [aload.tile([128, N], F32R, tag=f"a{mch}", name=f"a{p}_{mch}")
                          for mch in range(2)]
                    for mch in range(2):
                        nc.sync.dma_start(out=at[mch][:], in_=A_in[p, mch])
                    ats.append(at)
                    for c in range(LCH):
                        ps = bpsum.tile([128, N], F32, tag="ps", name=f"ps{p}_{c}")
                        for mch in range(2):
                            nc.tensor.matmul(
                                ps[:], at[mch][:, 128 * c:128 * (c + 1)], at[mch][:],
                                start=(mch == 0), stop=(mch == 1))
                        nc.scalar.copy(atabf[c][:, N * p:N * (p + 1)], ps[:])
                    if p % 4 == 3:
                        g = p // 4
                        bt = [aload.tile([128, 32], F32R, tag=f"b{mch}",
                                         name=f"bv{g}_{mch}") for mch in range(2)]
                        for mch in range(2):
                            nc.sync.dma_start(out=bt[mch][:], in_=bT_in[g, mch])
                        t, m = g // 4, g % 4
                        for j in range(4):
                            psb = bpsum.tile([8, N], F32, tag="psb",
                                             name=f"psb{g}_{j}")
                            for mch in range(2):
                                nc.tensor.matmul(
                                    psb[:], bt[mch][:, 8 * j:8 * j + 8],
                                    ats[4 * g + j][mch][:],
                                    start=(mch == 0), stop=(mch == 1))
                            bpad = bpadp.tile([8, N], F32, tag="bp",
                                              name=f"bpad{g}_{j}")
                            nc.scalar.copy(bpad[:], psb[:])
                            nc.sync.dma_start(
                                out=atb_t[t][32 * m + 8 * j:32 * m + 8 * j + 8, :],
                                in_=bpad[:])

            with (
                tc.tile_pool(name="tpsum", bufs=2, space="PSUM") as tpsum,
                tc.tile_pool(name="wpsum", bufs=3, space="PSUM") as wpsum,
                tc.tile_pool(name="dpsum", bufs=2, space="PSUM") as dpsum,
                tc.tile_pool(name="scr", bufs=2) as scr,
                tc.tile_pool(name="wpad", bufs=3) as wpadp,
            ):
                def make_yt(tag):
                    for t in range(NT):
                        for c in range(LCH):
                            tp = tpsum.tile([128, 128], F32, tag="tp",
                                            name=f"tp{tag}_{t}{c}")
                            nc.tensor.transpose(
                                tp[:], y_t[t][:, 128 * c:128 * (c + 1)], ident[:])
                            nc.vector.tensor_copy(
                                yt_c[c][:, 128 * t:128 * (t + 1)], tp[:])

                make_yt("init")

                for k in range(K):
                    # ---------- B-side: process y_k (k>=1) -> C_k -----------
                    if k >= 1:
                        nc.sync.dma_start(out=yf_t[0][:], in_=a2a1_out[0:128])
                        nc.sync.dma_start(out=yf_t[1][:], in_=a2a1_out[128:256])
                        c2s = []
                        for t in range(2):
                            c2 = scr.tile([128, N], F32, tag=f"c2{t}",
                                          name=f"c2{k}{t}")
                            nc.scalar.activation(c2[:], ub_t[t][:], AFT.Copy,
                                                 scale=degb(t))
                            c2s.append(c2)

                    # ---------- matvec: w = AtA @ y_k (col-tiled 4p/bank) ---
                    for g in range(NG):
                        ps = wpsum.tile([128, N], F32, tag="w", name=f"wps{k}_{g}")
                        for j in range(4):
                            p = 4 * g + j
                            for c in range(LCH):
                                nc.tensor.matmul(
                                    ps[32 * j:32 * j + 8, :],
                                    yt_c[c][:, 8 * p:8 * p + 8],
                                    atabf[c][:, N * p:N * (p + 1)],
                                    start=(c == 0), stop=(c == LCH - 1),
                                    tile_position=(0, 32 * j))
                        wpad = wpadp.tile([128, N], F32, tag="wp", name=f"wpad{k}_{g}")
                        nc.scalar.copy(wpad[:], ps[:])
                        t, m = g // 4, g % 4
                        for j in range(4):
                            nc.sync.dma_start(
                                out=w_t[t][32 * m + 8 * j:32 * m + 8 * j + 8, :],
                                in_=wpad[32 * j:32 * j + 8, :])
                        # interleave the B-side Adj matmuls mid-matvec so they
                        # run as soon as AllToAll #1 has landed
                        if k >= 1 and g == 5:
                            dps = [None, None]
                            for t2 in range(2):
                                dps[t2] = dpsum.tile([128, N], F32, tag="d",
                                                     name=f"dps{k}{t2}")
                                for q in range(2):
                                    nc.tensor.matmul(
                                        dps[t2][:],
                                        adj_t[q][:, 128 * t2:128 * (t2 + 1)],
                                        yf_t[q][:].bitcast(F32R),
                                        start=(q == 0), stop=(q == 1))

                    # ---------- grad pre-C part (overlaps matvec/B-side) ----
                    for t in range(NT):
                        sga = scr.tile([128, N], F32, tag=f"s{t}", name=f"s{k}{t}")
                        # sign(y)*tau via ACT Sign + per-partition tau scale
                        nc.scalar.activation(sga[:], y_t[t][:], AFT.Sign)
                        nc.vector.tensor_scalar_mul(sga[:], sga[:], taucol(k, t))
                        nc.vector.tensor_tensor(w_t[t][:], w_t[t][:], atb_t[t][:],
                                                TT.subtract)
                        nc.vector.tensor_tensor(w_t[t][:], w_t[t][:], sga[:], TT.add)

                    # ---------- B-side elementwise + AllToAll #2 ------------
                    if k >= 1:
                        kb = k - 1
                        dlts = []
                        for t in range(2):
                            dlt = scr.tile([128, N], F32, tag=f"dlt{t}",
                                           name=f"dlt{k}{t}")
                            cf = scr.tile([128, N], F32, tag=f"cf{t}",
                                          name=f"cf{k}{t}")
                            # delta = 2*deg*y - Adj@y
                            nc.vector.tensor_scalar_mul(dlt[:], yf_t[t][:], degb2(t))
                            nc.vector.tensor_tensor(dlt[:], dlt[:], dps[t][:],
                                                    TT.subtract)
                            dlts.append(dlt)
                            # C = U_{k-1}*deg + (eta*deg + rho)*delta
                            nc.vector.tensor_scalar_mul(cf[:], dlt[:], rhocol(kb, t))
                            nc.vector.tensor_tensor(cf[:], cf[:], c2s[t][:], TT.add)
                            nc.sync.dma_start(
                                out=a2a2_in[128 * t:128 * (t + 1)], in_=cf[:])
                        nc.gpsimd.collective_compute(
                            "AllToAll", TT.bypass, replica_groups=rg,
                            ins=[a2a2_in[:].opt()], outs=[a2a2_out[:].opt()])
                        for t in range(2):
                            tmpb = scr.tile([128, N], F32, tag=f"tmpb{t}",
                                            name=f"tmpb{k}{t}")
                            nc.scalar.activation(tmpb[:], dlts[t][:], AFT.Copy,
                                                 scale=etacol(kb, t))
                            nc.vector.tensor_tensor(ub_t[t][:], ub_t[t][:], tmpb[:],
                                                    TT.add)
                        for t in range(NT):
                            nc.sync.dma_start(
                                out=c_t[t][:],
                                in_=a2a2_out[:, PPT * t:PPT * (t + 1), :]
                                .rearrange("b p n -> p b n"))

                    # ---------- grad tail (after C arrives) -----------------
                    for t in range(NT):
                        nc.vector.tensor_tensor(w_t[t][:], w_t[t][:], c_t[t][:], TT.add)
                        nc.vector.tensor_scalar(
                            w_t[t][:], w_t[t][:], float(gmax[k]), float(-gmax[k]),
                            TT.min, TT.max)
                        nc.scalar.activation(w_t[t][:], w_t[t][:], AFT.Copy,
                                             scale=alpha(k, t))
                        nc.vector.tensor_tensor(y_t[t][:], y_t[t][:], w_t[t][:],
                                                TT.subtract)
                        if k < K - 1:
                            nc.sync.dma_start(
                                out=a2a1_in[:, PPT * t:PPT * (t + 1), :]
                                .rearrange("b p n -> p b n"),
                                in_=y_t[t][:])
                    if k < K - 1:
                        nc.gpsimd.collective_compute(
                            "AllToAll", TT.bypass, replica_groups=rg,
                            ins=[a2a1_in[:].opt()], outs=[a2a1_out[:].opt()])
                        make_yt(f"s{k}")
                    for t in range(NT):
                        nc.scalar.dma_start(
                            out=Y_out[k, :, PPT * t:PPT * (t + 1), :]
                            .rearrange("b p n -> p b n"),
                            in_=y_t[t][:])

    _fix_multiwait(nc)
    return nc


_NC_CACHE = {}


def kernel(A, b, param, max_param, y0, U0, delta0, edge_index):
    A = np.asarray(A, np.float32)
    b = np.asarray(b, np.float32)
    param = np.asarray(param, np.float32)
    max_param = np.asarray(max_param, np.float32)
    y0 = np.asarray(y0, np.float32)
    U0 = np.asarray(U0, np.float32)
    delta0 = np.asarray(delta0, np.float32)
    edge_index = np.asarray(edge_index, np.int32)

    hyp = 1.0 / (1.0 + np.exp(-np.cumsum(param.astype(np.float32), axis=0)))
    hyp = np.clip(hyp * max_param[None, None, :], 1e-4, 0.99).astype(np.float32)
    ks = np.arange(K, dtype=np.float32)
    gmax = np.maximum(1.0, 30.0 - ks)
    ymax = np.maximum(10.0, 200.0 - 3.0 * ks)

    deg = np.zeros((B, P), np.float32)
    Adj = np.zeros((B, P, P), np.float32)
    for bb in range(B):
        np.add.at(deg[bb], edge_index[bb, 0], 1.0)
        np.add.at(Adj[bb], (edge_index[bb, 0], edge_index[bb, 1]), 1.0)
        np.add.at(Adj[bb], (edge_index[bb, 1], edge_index[bb, 0]), 1.0)

    C0 = U0[..., 0] * deg[:, :, None] + delta0[..., 0] * hyp[0, :, 2][None, :, None]

    key = (gmax.tobytes(), ymax.tobytes())
    if key not in _NC_CACHE:
        _NC_CACHE[key] = _build_nc(gmax, ymax)
    nc = _NC_CACHE[key]

    def pshard(x, c):
        """[B, P, N] -> packed state layout [NT, 128, N], rows (p_l, b)."""
        sl = x[:, PL * c:PL * (c + 1), :]
        sl = np.transpose(sl, (1, 0, 2)).reshape(NT, 128, -1)
        return np.ascontiguousarray(sl)

    A0 = A[0]
    in_maps = []
    for c in range(NCORES):
        Asl = A0[PL * c:PL * (c + 1)].reshape(PL, 2, 128, N)
        # b vectors grouped 4 p's x 8 batches to match the col-tiled Atb build
        bsl = b[:, PL * c:PL * (c + 1), :, 0]            # [B, PL, M]
        bT = np.transpose(bsl, (2, 1, 0)).reshape(2, 128, NG, 4, B)
        bT = np.transpose(bT, (2, 0, 1, 3, 4)).reshape(NG, 2, 128, 32)

        # per-partition scalar columns
        scalp = np.zeros((128, 2 * K * NT), np.float32)
        for k in range(K):
            for t in range(NT):
                pg = PL * c + PPT * t + np.arange(128) // 8
                scalp[:, 2 * (k * NT + t)] = hyp[k, pg, 0]
                scalp[:, 2 * (k * NT + t) + 1] = hyp[k, pg, 1]
        scalb = np.zeros((128, 2 * (K - 1) * 2 + 4), np.float32)
        for kb in range(K - 1):
            for t in range(2):
                pg = 128 * t + np.arange(128)
                scalb[:, 2 * (kb * 2 + t)] = (hyp[kb + 1, pg, 2]
                                              + hyp[kb, pg, 3] * deg[c, pg])
                scalb[:, 2 * (kb * 2 + t) + 1] = hyp[kb, pg, 3]
        for t in range(2):
            scalb[:, 2 * (K - 1) * 2 + t] = 2.0 * deg[c, 128 * t:128 * (t + 1)]
            scalb[:, 2 * (K - 1) * 2 + 2 + t] = deg[c, 128 * t:128 * (t + 1)]

        in_maps.append({
            "A_in": np.ascontiguousarray(Asl),
            "bT_in": np.ascontiguousarray(bT),
            "y0_in": pshard(y0[..., 0], c),
            "c0_in": pshard(C0, c),
            "ub0_in": np.ascontiguousarray(U0[c, :, :, 0].reshape(2, 128, N)),
            "adj_in": np.ascontiguousarray(Adj[c].reshape(2, 128, P)),
            "scalp_in": scalp,
            "scalb_in": scalb,
            "ident_in": np.eye(128, dtype=np.float32),
        })

    res = run_bass_kernel_spmd(nc, in_maps, list(range(NCORES)))

    Y = np.empty((K, B, P, N, 1), np.float32)
    for c in range(NCORES):
        Y[:, :, PL * c:PL * (c + 1), :, 0] = res.results[c]["Y_out"]
    hyp_last = hyp[-1][..., None]
    return Y, hyp_last


# revision 18
# speedup vs baseline: 1.0901x; 1.0901x over previous
"""DLASSO unfolded GNN message-passing kernel for 8 Trainium2 NeuronCores.

Sharding: P (graph partitions / per-partition Gram matrices) split 32/core.
  - AtA built on device (fp32r matmuls), stored bf16 in SBUF (16 MB/core),
    streamed through the PE as the moving operand each step with the per-
    partition y vectors (8 batches) as an 8-column stationary operand.
    Four partitions share each PSUM bank via col-tiling (tile_position).
  - State tensors live in packed [16p*8b, 512] fp32 SBUF tiles; PSUM
    results are extracted with wide ACT copies into a 32-row-strided padded
    tile and re-laned with small partition-sliced SBUF->SBUF DMAs.
  - The graph scatter (delta = 2*deg*y - Adj@y) is batch-local: each step
    reshards y P->B (AllToAll #1), computes delta/U and C = U*deg+delta*rho
    on the batch-sharded side (core c owns batch c, Adj matmul in fp32r),
    and ships C back B->P (AllToAll #2), overlapping the next matvec.
"""

import sys

sys.path.insert(0, "/opt/trn_rl_repo")

import ml_dtypes
import numpy as np

import concourse.bass as bass
import concourse.mybir as mybir
import concourse.tile as tile
from concourse.bass_utils import run_bass_kernel_spmd

B, P, M, N, K = 8, 256, 256, 512, 15
NCORES = 8
PL = P // NCORES          # 32 partitions per core
NT = 2                    # packed state tiles per tensor
PPT = 16                  # p's per packed tile
NG = PL // 4              # 8 psum groups of 4 p's
LCH = 4                   # 128-row l-chunks in the contraction dim
F32 = mybir.dt.float32
F32R = mybir.dt.float32r
BF16 = mybir.dt.bfloat16
TT = mybir.AluOpType
AFT = mybir.ActivationFunctionType


def _fix_multiwait(nc):
    """walrus here caps sem-waits per instruction at 1 (2 for EventSemaphore);
    Tile's kernel-tail drain can carry more - hoist extras onto NOPs."""
    cnt = 0
    for f in nc.m.functions:
        for bb in f.blocks:
            out, changed = [], False
            for ins in bb.instructions:
                si = ins.sync_info
                cap = 2 if isinstance(ins, mybir.InstEventSemaphore) else 1
                if si is not None and len(si.on_wait) > cap:
                    waits = list(si.on_wait)
                    for w in waits[:-cap]:
                        cnt += 1
                        out.append(mybir.InstNoOp(
                            name=f"waitfix-{cnt}", engine=ins.engine,
                            bass_nofuse=True,
                            sync_info=mybir.SyncInfo(on_wait=[w], on_update=[])))
                    ins.sync_info = mybir.SyncInfo(
                        on_wait=waits[-cap:], on_update=list(si.on_update))
                    changed = True
                out.append(ins)
            if changed:
                bb.instructions = out
    return cnt


def _build_nc(gmax, ymax):
    nc = bass.Bass()

    A_in = nc.dram_tensor("A_in", [PL, 2, 128, N], F32R, kind="ExternalInput")
    bT_in = nc.dram_tensor("bT_in", [NG, 2, 128, 32], F32R, kind="ExternalInput")
    y0_in = nc.dram_tensor("y0_in", [NT, 128, N], F32, kind="ExternalInput")
    c0_in = nc.dram_tensor("c0_in", [NT, 128, N], F32, kind="ExternalInput")
    ub0_in = nc.dram_tensor("ub0_in", [2, 128, N], F32, kind="ExternalInput")
    adj_in = nc.dram_tensor("adj_in", [2, 128, P], F32R, kind="ExternalInput")
    # per-partition scalar columns: [128, x]
    scalp_in = nc.dram_tensor("scalp_in", [128, 2 * K * NT], F32, kind="ExternalInput")
    scalb_in = nc.dram_tensor("scalb_in", [128, 2 * (K - 1) * 2 + 4], F32,
                              kind="ExternalInput")
    ident_in = nc.dram_tensor("ident_in", [128, 128], F32, kind="ExternalInput")
    Y_out = nc.dram_tensor("Y_out", [K, B, PL, N], F32, kind="ExternalOutput")

    rg = [list(range(NCORES))]

    with tile.TileContext(nc) as tc:
        with (
            tc.tile_pool(name="const", bufs=1) as constp,
            tc.tile_pool(name="ata", bufs=1) as atap,
            tc.tile_pool(name="state", bufs=1) as statep,
            tc.tile_pool(name="dram", bufs=1, space="DRAM") as dram,
        ):
            atabf = [atap.tile([128, PL * N], BF16, name=f"atabf{c}") for c in range(LCH)]
            y_t = [statep.tile([128, N], F32, name=f"y{t}") for t in range(NT)]
            w_t = [statep.tile([128, N], F32, name=f"w{t}") for t in range(NT)]
            atb_t = [statep.tile([128, N], F32, name=f"atb{t}") for t in range(NT)]
            c_t = [statep.tile([128, N], F32, name=f"c{t}") for t in range(NT)]
            ub_t = [statep.tile([128, N], F32, name=f"ub{t}") for t in range(NT)]
            yf_t = [statep.tile([128, N], F32, name=f"yf{t}") for t in range(2)]
            yt_c = [statep.tile([128, PL * B], BF16, name=f"yt{c}") for c in range(LCH)]
            adj_t = [constp.tile([128, P], F32R, name=f"adj{q}") for q in range(2)]
            scalp = constp.tile([128, 2 * K * NT], F32, name="scalp")
            scalb = constp.tile([128, 2 * (K - 1) * 2 + 4], F32, name="scalb")
            ident = constp.tile([128, 128], F32, name="ident")

            def alpha(k, t):  # [128,1] per-partition alpha_k rows (p,b)
                return scalp[:, 2 * (k * NT + t):2 * (k * NT + t) + 1]

            def taucol(k, t):
                return scalp[:, 2 * (k * NT + t) + 1:2 * (k * NT + t) + 2]

            def rhocol(kb, t):  # B-side rows = p; rho_{kb+1}
                return scalb[:, 2 * (kb * 2 + t):2 * (kb * 2 + t) + 1]

            def etacol(kb, t):
                return scalb[:, 2 * (kb * 2 + t) + 1:2 * (kb * 2 + t) + 2]

            def degb2(t):
                return scalb[:, 2 * (K - 1) * 2 + t:2 * (K - 1) * 2 + t + 1]

            def degb(t):
                return scalb[:, 2 * (K - 1) * 2 + 2 + t:2 * (K - 1) * 2 + 3 + t]

            for q in range(2):
                nc.sync.dma_start(out=adj_t[q][:], in_=adj_in[q])
            nc.sync.dma_start(out=scalp[:], in_=scalp_in[:])
            nc.sync.dma_start(out=scalb[:], in_=scalb_in[:])
            nc.sync.dma_start(out=ident[:], in_=ident_in[:])
            for t in range(NT):
                nc.sync.dma_start(out=y_t[t][:], in_=y0_in[t])
                nc.sync.dma_start(out=c_t[t][:], in_=c0_in[t])
                nc.sync.dma_start(out=ub_t[t][:], in_=ub0_in[t])

            a2a1_in = dram.tile([B, PL, N], F32, name="a2a1_in")
            a2a1_out = dram.tile([P, N], F32, name="a2a1_out")
            a2a2_in = dram.tile([P, N], F32, name="a2a2_in")
            a2a2_out = dram.tile([B, PL, N], F32, name="a2a2_out")

            # ---- setup: AtA (bf16) and Atb on device ------------------------
            with (
                tc.tile_pool(name="aload", bufs=5) as aload,
                tc.tile_pool(name="bpsum", bufs=4, space="PSUM") as bpsum,
                tc.tile_pool(name="bpad", bufs=2) as bpadp,
            ):
                ats = []
                for p in range(PL):
                    at = [aload.tile([128, N], F32R, tag=f"a{mch}", name=f"a{p}_{mch}")
                          for mch in range(2)]
                    for mch in range(2):
                        nc.sync.dma_start(out=at[mch][:], in_=A_in[p, mch])
                    ats.append(at)
                    for c in range(LCH):
                        ps = bpsum.tile([128, N], F32, tag="ps", name=f"ps{p}_{c}")
                        for mch in range(2):
                            nc.tensor.matmul(
                                ps[:], at[mch][:, 128 * c:128 * (c + 1)], at[mch][:],
                                start=(mch == 0), stop=(mch == 1))
                        nc.scalar.copy(atabf[c][:, N * p:N * (p + 1)], ps[:])
                    if p % 4 == 3:
                        g = p // 4
                        bt = [aload.tile([128, 32], F32R, tag=f"b{mch}",
                                         name=f"bv{g}_{mch}") for mch in range(2)]
                        for mch in range(2):
                            nc.sync.dma_start(out=bt[mch][:], in_=bT_in[g, mch])
                        t, m = g // 4, g % 4
                        for j in range(4):
                            psb = bpsum.tile([8, N], F32, tag="psb",
                                             name=f"psb{g}_{j}")
                            for mch in range(2):
                                nc.tensor.matmul(
                                    psb[:], bt[mch][:, 8 * j:8 * j + 8],
                                    ats[4 * g + j][mch][:],
                                    start=(mch == 0), stop=(mch == 1))
                            bpad = bpadp.tile([8, N], F32, tag="bp",
                                              name=f"bpad{g}_{j}")
                            nc.scalar.copy(bpad[:], psb[:])
                            nc.sync.dma_start(
                                out=atb_t[t][32 * m + 8 * j:32 * m + 8 * j + 8, :],
                                in_=bpad[:])

            with (
                tc.tile_pool(name="tpsum", bufs=3, space="PSUM") as tpsum,
                tc.tile_pool(name="wpsum", bufs=3, space="PSUM") as wpsum,
                tc.tile_pool(name="dpsum", bufs=2, space="PSUM") as dpsum,
                tc.tile_pool(name="scr", bufs=2) as scr,
                tc.tile_pool(name="wpad", bufs=3) as wpadp,
            ):
                def make_yt(tag):
                    for t in range(NT):
                        for c in range(LCH):
                            tp = tpsum.tile([128, 128], F32, tag="tp",
                                            name=f"tp{tag}_{t}{c}")
                            nc.tensor.transpose(
                                tp[:], y_t[t][:, 128 * c:128 * (c + 1)], ident[:])
                            nc.vector.tensor_copy(
                                yt_c[c][:, 128 * t:128 * (t + 1)], tp[:])

                make_yt("init")

                for k in range(K):
                    # ---------- B-side: process y_k (k>=1) -> C_k -----------
                    if k >= 1:
                        nc.sync.dma_start(out=yf_t[0][:], in_=a2a1_out[0:128])
                        nc.sync.dma_start(out=yf_t[1][:], in_=a2a1_out[128:256])
                        c2s = []
                        for t in range(2):
                            c2 = scr.tile([128, N], F32, tag=f"c2{t}",
                                          name=f"c2{k}{t}")
                            nc.scalar.activation(c2[:], ub_t[t][:], AFT.Copy,
                                                 scale=degb(t))
                            c2s.append(c2)

                    # ---------- matvec: w = AtA @ y_k (col-tiled 4p/bank) ---
                    for g in range(NG):
                        ps = wpsum.tile([128, N], F32, tag="w", name=f"wps{k}_{g}")
                        for j in range(4):
                            p = 4 * g + j
                            for c in range(LCH):
                                nc.tensor.matmul(
                                    ps[32 * j:32 * j + 8, :],
                                    yt_c[c][:, 8 * p:8 * p + 8],
                                    atabf[c][:, N * p:N * (p + 1)],
                                    start=(c == 0), stop=(c == LCH - 1),
                                    tile_position=(0, 32 * j))
                        wpad = wpadp.tile([128, N], F32, tag="wp", name=f"wpad{k}_{g}")
                        nc.scalar.copy(wpad[:], ps[:])
                        t, m = g // 4, g % 4
                        for j in range(4):
                            nc.sync.dma_start(
                                out=w_t[t][32 * m + 8 * j:32 * m + 8 * j + 8, :],
                                in_=wpad[32 * j:32 * j + 8, :])
                        # interleave the B-side Adj matmuls mid-matvec so they
                        # run as soon as AllToAll #1 has landed
                        if k >= 1 and g == 5:
                            dps = [None, None]
                            for t2 in range(2):
                                dps[t2] = dpsum.tile([128, N], F32, tag="d",
                                                     name=f"dps{k}{t2}")
                                for q in range(2):
                                    nc.tensor.matmul(
                                        dps[t2][:],
                                        adj_t[q][:, 128 * t2:128 * (t2 + 1)],
                                        yf_t[q][:].bitcast(F32R),
                                        start=(q == 0), stop=(q == 1))

                    # ---------- grad pre-C part (overlaps matvec/B-side) ----
                    for t in range(NT):
                        sga = scr.tile([128, N], F32, tag=f"s{t}", name=f"s{k}{t}")
                        # sign(y)*tau via ACT Sign + per-partition tau scale
                        nc.scalar.activation(sga[:], y_t[t][:], AFT.Sign)
                        nc.vector.tensor_scalar_mul(sga[:], sga[:], taucol(k, t))
                        nc.vector.tensor_tensor(w_t[t][:], w_t[t][:], atb_t[t][:],
                                                TT.subtract)
                        nc.vector.tensor_tensor(w_t[t][:], w_t[t][:], sga[:], TT.add)

                    # ---------- B-side elementwise + AllToAll #2 ------------
                    if k >= 1:
                        kb = k - 1
                        dlts = []
                        for t in range(2):
                            dlt = scr.tile([128, N], F32, tag=f"dlt{t}",
                                           name=f"dlt{k}{t}")
                            cf = scr.tile([128, N], F32, tag=f"cf{t}",
                                          name=f"cf{k}{t}")
                            # delta = 2*deg*y - Adj@y
                            nc.vector.tensor_scalar_mul(dlt[:], yf_t[t][:], degb2(t))
                            nc.vector.tensor_tensor(dlt[:], dlt[:], dps[t][:],
                                                    TT.subtract)
                            dlts.append(dlt)
                            # C = U_{k-1}*deg + (eta*deg + rho)*delta
                            nc.vector.tensor_scalar_mul(cf[:], dlt[:], rhocol(kb, t))
                            nc.vector.tensor_tensor(cf[:], cf[:], c2s[t][:], TT.add)
                            nc.sync.dma_start(
                                out=a2a2_in[128 * t:128 * (t + 1)], in_=cf[:])
                        nc.gpsimd.collective_compute(
                            "AllToAll", TT.bypass, replica_groups=rg,
                            ins=[a2a2_in[:].opt()], outs=[a2a2_out[:].opt()])
                        for t in range(2):
                            tmpb = scr.tile([128, N], F32, tag=f"tmpb{t}",
                                            name=f"tmpb{k}{t}")
                            nc.scalar.activation(tmpb[:], dlts[t][:], AFT.Copy,
                                                 scale=etacol(kb, t))
                            nc.vector.tensor_tensor(ub_t[t][:], ub_t[t][:], tmpb[:],
                                                    TT.add)
                        for t in range(NT):
                            nc.sync.dma_start(
                                out=c_t[t][:],
                                in_=a2a2_out[:, PPT * t:PPT * (t + 1), :]
                                .rearrange("b p n -> p b n"))

                    # ---------- grad tail (after C arrives) -----------------
                    for t in range(NT):
                        nc.vector.tensor_tensor(w_t[t][:], w_t[t][:], c_t[t][:], TT.add)
                        nc.vector.tensor_scalar(
                            w_t[t][:], w_t[t][:], float(gmax[k]), float(-gmax[k]),
                            TT.min, TT.max)
                        nc.scalar.activation(w_t[t][:], w_t[t][:], AFT.Copy,
                                             scale=alpha(k, t))
                        nc.vector.tensor_tensor(y_t[t][:], y_t[t][:], w_t[t][:],
                                                TT.subtract)
                        if k < K - 1:
                            nc.sync.dma_start(
                                out=a2a1_in[:, PPT * t:PPT * (t + 1), :]
                                .rearrange("b p n -> p b n"),
                                in_=y_t[t][:])
                    if k < K - 1:
                        nc.gpsimd.collective_compute(
                            "AllToAll", TT.bypass, replica_groups=rg,
                            ins=[a2a1_in[:].opt()], outs=[a2a1_out[:].opt()])
                        make_yt(f"s{k}")
                    for t in range(NT):
                        nc.scalar.dma_start(
                            out=Y_out[k, :, PPT * t:PPT * (t + 1), :]
                            .rearrange("b p n -> p b n"),
                            in_=y_t[t][:])

    _fix_multiwait(nc)
    return nc


_NC_CACHE = {}


def kernel(A, b, param, max_param, y0, U0, delta0, edge_index):
    A = np.asarray(A, np.float32)
    b = np.asarray(b, np.float32)
    param = np.asarray(param, np.float32)
    max_param = np.asarray(max_param, np.float32)
    y0 = np.asarray(y0, np.float32)
    U0 = np.asarray(U0, np.float32)
    delta0 = np.asarray(delta0, np.float32)
    edge_index = np.asarray(edge_index, np.int32)

    hyp = 1.0 / (1.0 + np.exp(-np.cumsum(param.astype(np.float32), axis=0)))
    hyp = np.clip(hyp * max_param[None, None, :], 1e-4, 0.99).astype(np.float32)
    ks = np.arange(K, dtype=np.float32)
    gmax = np.maximum(1.0, 30.0 - ks)
    ymax = np.maximum(10.0, 200.0 - 3.0 * ks)

    deg = np.zeros((B, P), np.float32)
    Adj = np.zeros((B, P, P), np.float32)
    for bb in range(B):
        np.add.at(deg[bb], edge_index[bb, 0], 1.0)
        np.add.at(Adj[bb], (edge_index[bb, 0], edge_index[bb, 1]), 1.0)
        np.add.at(Adj[bb], (edge_index[bb, 1], edge_index[bb, 0]), 1.0)

    C0 = U0[..., 0] * deg[:, :, None] + delta0[..., 0] * hyp[0, :, 2][None, :, None]

    key = (gmax.tobytes(), ymax.tobytes())
    if key not in _NC_CACHE:
        _NC_CACHE[key] = _build_nc(gmax, ymax)
    nc = _NC_CACHE[key]

    def pshard(x, c):
        """[B, P, N] -> packed state layout [NT, 128, N], rows (p_l, b)."""
        sl = x[:, PL * c:PL * (c + 1), :]
        sl = np.transpose(sl, (1, 0, 2)).reshape(NT, 128, -1)
        return np.ascontiguousarray(sl)

    A0 = A[0]
    in_maps = []
    for c in range(NCORES):
        Asl = A0[PL * c:PL * (c + 1)].reshape(PL, 2, 128, N)
        # b vectors grouped 4 p's x 8 batches to match the col-tiled Atb build
        bsl = b[:, PL * c:PL * (c + 1), :, 0]            # [B, PL, M]
        bT = np.transpose(bsl, (2, 1, 0)).reshape(2, 128, NG, 4, B)
        bT = np.transpose(bT, (2, 0, 1, 3, 4)).reshape(NG, 2, 128, 32)

        # per-partition scalar columns
        scalp = np.zeros((128, 2 * K * NT), np.float32)
        for k in range(K):
            for t in range(NT):
                pg = PL * c + PPT * t + np.arange(128) // 8
                scalp[:, 2 * (k * NT + t)] = hyp[k, pg, 0]
                scalp[:, 2 * (k * NT + t) + 1] = hyp[k, pg, 1]
        scalb = np.zeros((128, 2 * (K - 1) * 2 + 4), np.float32)
        for kb in range(K - 1):
            for t in range(2):
                pg = 128 * t + np.arange(128)
                scalb[:, 2 * (kb * 2 + t)] = (hyp[kb + 1, pg, 2]
                                              + hyp[kb, pg, 3] * deg[c, pg])
                scalb[:, 2 * (kb * 2 + t) + 1] = hyp[kb, pg, 3]
        for t in range(2):
            scalb[:, 2 * (K - 1) * 2 + t] = 2.0 * deg[c, 128 * t:128 * (t + 1)]
            scalb[:, 2 * (K - 1) * 2 + 2 + t] = deg[c, 128 * t:128 * (t + 1)]

        in_maps.append({
            "A_in": np.ascontiguousarray(Asl),
            "bT_in": np.ascontiguousarray(bT),
            "y0_in": pshard(y0[..., 0], c),
            "c0_in": pshard(C0, c),
            "ub0_in": np.ascontiguousarray(U0[c, :, :, 0].reshape(2, 128, N)),
            "adj_in": np.ascontiguousarray(Adj[c].reshape(2, 128, P)),
            "scalp_in": scalp,
            "scalb_in": scalb,
            "ident_in": np.eye(128, dtype=np.float32),
        })

    res = run_bass_kernel_spmd(nc, in_maps, list(range(NCORES)))

    Y = np.empty((K, B, P, N, 1), np.float32)
    for c in range(NCORES):
        Y[:, :, PL * c:PL * (c + 1), :, 0] = res.results[c]["Y_out"]
    hyp_last = hyp[-1][..., None]
    return Y, hyp_last


# revision 19
# speedup vs baseline: 1.1591x; 1.0633x over previous
"""DLASSO unfolded GNN message-passing kernel for 8 Trainium2 NeuronCores.

Sharding: P (graph partitions / per-partition Gram matrices) split 32/core.
  - AtA built on device (fp32r matmuls), stored bf16 in SBUF (16 MB/core),
    streamed through the PE as the moving operand each step with the per-
    partition y vectors (8 batches) as an 8-column stationary operand.
    Four partitions share each PSUM bank via col-tiling (tile_position).
  - State tensors live in packed [16p*8b, 512] fp32 SBUF tiles; PSUM
    results are extracted with wide ACT copies into a 32-row-strided padded
    tile and re-laned with small partition-sliced SBUF->SBUF DMAs.
  - The graph scatter (delta = 2*deg*y - Adj@y) is batch-local: each step
    reshards y P->B (AllToAll #1), computes delta/U and C = U*deg+delta*rho
    on the batch-sharded side (core c owns batch c, Adj matmul in fp32r),
    and ships C back B->P (AllToAll #2), overlapping the next matvec.
"""

import sys

sys.path.insert(0, "/opt/trn_rl_repo")

import ml_dtypes
import numpy as np

import concourse.bass as bass
import concourse.mybir as mybir
import concourse.tile as tile
from concourse.bass_utils import run_bass_kernel_spmd

B, P, M, N, K = 8, 256, 256, 512, 15
NCORES = 8
PL = P // NCORES          # 32 partitions per core
NT = 2                    # packed state tiles per tensor
PPT = 16                  # p's per packed tile
NG = PL // 4              # 8 psum groups of 4 p's
LCH = 4                   # 128-row l-chunks in the contraction dim
F32 = mybir.dt.float32
F32R = mybir.dt.float32r
BF16 = mybir.dt.bfloat16
TT = mybir.AluOpType
AFT = mybir.ActivationFunctionType


def _fix_multiwait(nc):
    """walrus here caps sem-waits per instruction at 1 (2 for EventSemaphore);
    Tile's kernel-tail drain can carry more - hoist extras onto NOPs."""
    cnt = 0
    for f in nc.m.functions:
        for bb in f.blocks:
            out, changed = [], False
            for ins in bb.instructions:
                si = ins.sync_info
                cap = 2 if isinstance(ins, mybir.InstEventSemaphore) else 1
                if si is not None and len(si.on_wait) > cap:
                    waits = list(si.on_wait)
                    for w in waits[:-cap]:
                        cnt += 1
                        out.append(mybir.InstNoOp(
                            name=f"waitfix-{cnt}", engine=ins.engine,
                            bass_nofuse=True,
                            sync_info=mybir.SyncInfo(on_wait=[w], on_update=[])))
                    ins.sync_info = mybir.SyncInfo(
                        on_wait=waits[-cap:], on_update=list(si.on_update))
                    changed = True
                out.append(ins)
            if changed:
                bb.instructions = out
    return cnt


def _build_nc(gmax, ymax):
    nc = bass.Bass()

    A_in = nc.dram_tensor("A_in", [PL, 2, 128, N], F32R, kind="ExternalInput")
    bT_in = nc.dram_tensor("bT_in", [NG, 2, 128, 32], F32R, kind="ExternalInput")
    y0_in = nc.dram_tensor("y0_in", [NT, 128, N], F32, kind="ExternalInput")
    c0_in = nc.dram_tensor("c0_in", [NT, 128, N], F32, kind="ExternalInput")
    ub0_in = nc.dram_tensor("ub0_in", [2, 128, N], F32, kind="ExternalInput")
    adj_in = nc.dram_tensor("adj_in", [2, 128, P], F32R, kind="ExternalInput")
    # per-partition scalar columns: [128, x]
    scalp_in = nc.dram_tensor("scalp_in", [128, 2 * K * NT], F32, kind="ExternalInput")
    scalb_in = nc.dram_tensor("scalb_in", [128, 2 * (K - 1) * 2 + 4], F32,
                              kind="ExternalInput")
    ident_in = nc.dram_tensor("ident_in", [128, 128], F32, kind="ExternalInput")
    Y_out = nc.dram_tensor("Y_out", [K, B, PL, N], F32, kind="ExternalOutput")

    rg = [list(range(NCORES))]

    with tile.TileContext(nc) as tc:
        with (
            tc.tile_pool(name="const", bufs=1) as constp,
            tc.tile_pool(name="ata", bufs=1) as atap,
            tc.tile_pool(name="state", bufs=1) as statep,
            tc.tile_pool(name="dram", bufs=1, space="DRAM") as dram,
        ):
            atabf = [atap.tile([128, PL * N], BF16, name=f"atabf{c}") for c in range(LCH)]
            y_t = [statep.tile([128, N], F32, name=f"y{t}") for t in range(NT)]
            w_t = [statep.tile([128, N], F32, name=f"w{t}") for t in range(NT)]
            atb_t = [statep.tile([128, N], F32, name=f"atb{t}") for t in range(NT)]
            c_t = [statep.tile([128, N], F32, name=f"c{t}") for t in range(NT)]
            ub_t = [statep.tile([128, N], F32, name=f"ub{t}") for t in range(NT)]
            yf_t = [statep.tile([128, N], F32, name=f"yf{t}") for t in range(2)]
            yt_c = [statep.tile([128, PL * B], BF16, name=f"yt{c}") for c in range(LCH)]
            adj_t = [constp.tile([128, P], F32R, name=f"adj{q}") for q in range(2)]
            scalp = constp.tile([128, 2 * K * NT], F32, name="scalp")
            scalb = constp.tile([128, 2 * (K - 1) * 2 + 4], F32, name="scalb")
            ident = constp.tile([128, 128], F32, name="ident")

            def alpha(k, t):  # [128,1] per-partition alpha_k rows (p,b)
                return scalp[:, 2 * (k * NT + t):2 * (k * NT + t) + 1]

            def taucol(k, t):
                return scalp[:, 2 * (k * NT + t) + 1:2 * (k * NT + t) + 2]

            def rhocol(kb, t):  # B-side rows = p; rho_{kb+1}
                return scalb[:, 2 * (kb * 2 + t):2 * (kb * 2 + t) + 1]

            def etacol(kb, t):
                return scalb[:, 2 * (kb * 2 + t) + 1:2 * (kb * 2 + t) + 2]

            def degb2(t):
                return scalb[:, 2 * (K - 1) * 2 + t:2 * (K - 1) * 2 + t + 1]

            def degb(t):
                return scalb[:, 2 * (K - 1) * 2 + 2 + t:2 * (K - 1) * 2 + 3 + t]

            for q in range(2):
                nc.sync.dma_start(out=adj_t[q][:], in_=adj_in[q])
            nc.sync.dma_start(out=scalp[:], in_=scalp_in[:])
            nc.sync.dma_start(out=scalb[:], in_=scalb_in[:])
            nc.sync.dma_start(out=ident[:], in_=ident_in[:])
            for t in range(NT):
                nc.sync.dma_start(out=y_t[t][:], in_=y0_in[t])
                nc.sync.dma_start(out=c_t[t][:], in_=c0_in[t])
                nc.sync.dma_start(out=ub_t[t][:], in_=ub0_in[t])

            a2a1_in = dram.tile([B, PL, N], F32, name="a2a1_in")
            a2a1_out = dram.tile([P, N], F32, name="a2a1_out")
            a2a2_in = dram.tile([P, N], F32, name="a2a2_in")
            a2a2_out = dram.tile([B, PL, N], F32, name="a2a2_out")

            # ---- setup: AtA (bf16) and Atb on device ------------------------
            with (
                tc.tile_pool(name="aload", bufs=5) as aload,
                tc.tile_pool(name="bpsum", bufs=4, space="PSUM") as bpsum,
                tc.tile_pool(name="bpad", bufs=2) as bpadp,
            ):
                ats = []
                for p in range(PL):
                    at = [aload.tile([128, N], F32R, tag=f"a{mch}", name=f"a{p}_{mch}")
                          for mch in range(2)]
                    for mch in range(2):
                        nc.sync.dma_start(out=at[mch][:], in_=A_in[p, mch])
                    ats.append(at)
                    for c in range(LCH):
                        ps = bpsum.tile([128, N], F32, tag="ps", name=f"ps{p}_{c}")
                        for mch in range(2):
                            nc.tensor.matmul(
                                ps[:], at[mch][:, 128 * c:128 * (c + 1)], at[mch][:],
                                start=(mch == 0), stop=(mch == 1))
                        nc.scalar.copy(atabf[c][:, N * p:N * (p + 1)], ps[:])
                    if p % 4 == 3:
                        g = p // 4
                        bt = [aload.tile([128, 32], F32R, tag=f"b{mch}",
                                         name=f"bv{g}_{mch}") for mch in range(2)]
                        for mch in range(2):
                            nc.sync.dma_start(out=bt[mch][:], in_=bT_in[g, mch])
                        t, m = g // 4, g % 4
                        for j in range(4):
                            psb = bpsum.tile([8, N], F32, tag="psb",
                                             name=f"psb{g}_{j}")
                            for mch in range(2):
                                nc.tensor.matmul(
                                    psb[:], bt[mch][:, 8 * j:8 * j + 8],
                                    ats[4 * g + j][mch][:],
                                    start=(mch == 0), stop=(mch == 1))
                            bpad = bpadp.tile([8, N], F32, tag="bp",
                                              name=f"bpad{g}_{j}")
                            nc.scalar.copy(bpad[:], psb[:])
                            nc.sync.dma_start(
                                out=atb_t[t][32 * m + 8 * j:32 * m + 8 * j + 8, :],
                                in_=bpad[:])

            with (
                tc.tile_pool(name="tpsum", bufs=3, space="PSUM") as tpsum,
                tc.tile_pool(name="wpsum", bufs=3, space="PSUM") as wpsum,
                tc.tile_pool(name="dpsum", bufs=2, space="PSUM") as dpsum,
                tc.tile_pool(name="scr", bufs=2) as scr,
                tc.tile_pool(name="wpad", bufs=4) as wpadp,
            ):
                def make_yt(tag):
                    for t in range(NT):
                        for c in range(LCH):
                            tp = tpsum.tile([128, 128], F32, tag="tp",
                                            name=f"tp{tag}_{t}{c}")
                            nc.tensor.transpose(
                                tp[:], y_t[t][:, 128 * c:128 * (c + 1)], ident[:])
                            nc.vector.tensor_copy(
                                yt_c[c][:, 128 * t:128 * (t + 1)], tp[:])

                make_yt("init")

                for k in range(K):
                    # ---------- B-side: process y_k (k>=1) -> C_k -----------
                    if k >= 1:
                        nc.sync.dma_start(out=yf_t[0][:], in_=a2a1_out[0:128])
                        nc.sync.dma_start(out=yf_t[1][:], in_=a2a1_out[128:256])
                        c2s = []
                        for t in range(2):
                            c2 = scr.tile([128, N], F32, tag=f"c2{t}",
                                          name=f"c2{k}{t}")
                            nc.scalar.activation(c2[:], ub_t[t][:], AFT.Copy,
                                                 scale=degb(t))
                            c2s.append(c2)

                    # ---------- matvec: w = AtA @ y_k (col-tiled 4p/bank) ---
                    for g in range(NG):
                        ps = wpsum.tile([128, N], F32, tag="w", name=f"wps{k}_{g}")
                        for j in range(4):
                            p = 4 * g + j
                            for c in range(LCH):
                                nc.tensor.matmul(
                                    ps[32 * j:32 * j + 8, :],
                                    yt_c[c][:, 8 * p:8 * p + 8],
                                    atabf[c][:, N * p:N * (p + 1)],
                                    start=(c == 0), stop=(c == LCH - 1),
                                    tile_position=(0, 32 * j))
                        wpad = wpadp.tile([128, N], F32, tag="wp", name=f"wpad{k}_{g}")
                        nc.scalar.copy(wpad[:], ps[:])
                        t, m = g // 4, g % 4
                        for j in range(4):
                            nc.scalar.dma_start(
                                out=w_t[t][32 * m + 8 * j:32 * m + 8 * j + 8, :],
                                in_=wpad[32 * j:32 * j + 8, :])
                        # interleave the B-side Adj matmuls mid-matvec so they
                        # run as soon as AllToAll #1 has landed
                        if k >= 1 and g == 5:
                            dps = [None, None]
                            for t2 in range(2):
                                dps[t2] = dpsum.tile([128, N], F32, tag="d",
                                                     name=f"dps{k}{t2}")
                                for q in range(2):
                                    nc.tensor.matmul(
                                        dps[t2][:],
                                        adj_t[q][:, 128 * t2:128 * (t2 + 1)],
                                        yf_t[q][:].bitcast(F32R),
                                        start=(q == 0), stop=(q == 1))

                    # ---------- grad pre-C part (overlaps matvec/B-side) ----
                    for t in range(NT):
                        sga = scr.tile([128, N], F32, tag=f"s{t}", name=f"s{k}{t}")
                        # sign(y)*tau via ACT Sign + per-partition tau scale
                        nc.scalar.activation(sga[:], y_t[t][:], AFT.Sign)
                        nc.vector.tensor_scalar_mul(sga[:], sga[:], taucol(k, t))
                        nc.vector.tensor_tensor(w_t[t][:], w_t[t][:], atb_t[t][:],
                                                TT.subtract)
                        nc.vector.tensor_tensor(w_t[t][:], w_t[t][:], sga[:], TT.add)

                    # ---------- B-side elementwise + AllToAll #2 ------------
                    if k >= 1:
                        kb = k - 1
                        dlts = []
                        for t in range(2):
                            dlt = scr.tile([128, N], F32, tag=f"dlt{t}",
                                           name=f"dlt{k}{t}")
                            cf = scr.tile([128, N], F32, tag=f"cf{t}",
                                          name=f"cf{k}{t}")
                            # delta = 2*deg*y - Adj@y
                            nc.vector.tensor_scalar_mul(dlt[:], yf_t[t][:], degb2(t))
                            nc.vector.tensor_tensor(dlt[:], dlt[:], dps[t][:],
                                                    TT.subtract)
                            dlts.append(dlt)
                            # C = U_{k-1}*deg + (eta*deg + rho)*delta
                            nc.vector.tensor_scalar_mul(cf[:], dlt[:], rhocol(kb, t))
                            nc.vector.tensor_tensor(cf[:], cf[:], c2s[t][:], TT.add)
                            nc.sync.dma_start(
                                out=a2a2_in[128 * t:128 * (t + 1)], in_=cf[:])
                        nc.gpsimd.collective_compute(
                            "AllToAll", TT.bypass, replica_groups=rg,
                            ins=[a2a2_in[:].opt()], outs=[a2a2_out[:].opt()])
                        for t in range(2):
                            tmpb = scr.tile([128, N], F32, tag=f"tmpb{t}",
                                            name=f"tmpb{k}{t}")
                            nc.scalar.activation(tmpb[:], dlts[t][:], AFT.Copy,
                                                 scale=etacol(kb, t))
                            nc.vector.tensor_tensor(ub_t[t][:], ub_t[t][:], tmpb[:],
                                                    TT.add)
                        for t in range(NT):
                            nc.sync.dma_start(
                                out=c_t[t][:],
                                in_=a2a2_out[:, PPT * t:PPT * (t + 1), :]
                                .rearrange("b p n -> p b n"))

                    # ---------- grad tail (after C arrives) -----------------
                    for t in range(NT):
                        nc.vector.tensor_tensor(w_t[t][:], w_t[t][:], c_t[t][:], TT.add)
                        nc.vector.tensor_scalar(
                            w_t[t][:], w_t[t][:], float(gmax[k]), float(-gmax[k]),
                            TT.min, TT.max)
                        nc.scalar.activation(w_t[t][:], w_t[t][:], AFT.Copy,
                                             scale=alpha(k, t))
                        nc.vector.tensor_tensor(y_t[t][:], y_t[t][:], w_t[t][:],
                                                TT.subtract)
                        if k < K - 1:
                            nc.sync.dma_start(
                                out=a2a1_in[:, PPT * t:PPT * (t + 1), :]
                                .rearrange("b p n -> p b n"),
                                in_=y_t[t][:])
                    if k < K - 1:
                        nc.gpsimd.collective_compute(
                            "AllToAll", TT.bypass, replica_groups=rg,
                            ins=[a2a1_in[:].opt()], outs=[a2a1_out[:].opt()])
                        make_yt(f"s{k}")
                    for t in range(NT):
                        nc.scalar.dma_start(
                            out=Y_out[k, :, PPT * t:PPT * (t + 1), :]
                            .rearrange("b p n -> p b n"),
                            in_=y_t[t][:])

    _fix_multiwait(nc)
    return nc


_NC_CACHE = {}


def kernel(A, b, param, max_param, y0, U0, delta0, edge_index):
    A = np.asarray(A, np.float32)
    b = np.asarray(b, np.float32)
    param = np.asarray(param, np.float32)
    max_param = np.asarray(max_param, np.float32)
    y0 = np.asarray(y0, np.float32)
    U0 = np.asarray(U0, np.float32)
    delta0 = np.asarray(delta0, np.float32)
    edge_index = np.asarray(edge_index, np.int32)

    hyp = 1.0 / (1.0 + np.exp(-np.cumsum(param.astype(np.float32), axis=0)))
    hyp = np.clip(hyp * max_param[None, None, :], 1e-4, 0.99).astype(np.float32)
    ks = np.arange(K, dtype=np.float32)
    gmax = np.maximum(1.0, 30.0 - ks)
    ymax = np.maximum(10.0, 200.0 - 3.0 * ks)

    deg = np.zeros((B, P), np.float32)
    Adj = np.zeros((B, P, P), np.float32)
    for bb in range(B):
        np.add.at(deg[bb], edge_index[bb, 0], 1.0)
        np.add.at(Adj[bb], (edge_index[bb, 0], edge_index[bb, 1]), 1.0)
        np.add.at(Adj[bb], (edge_index[bb, 1], edge_index[bb, 0]), 1.0)

    C0 = U0[..., 0] * deg[:, :, None] + delta0[..., 0] * hyp[0, :, 2][None, :, None]

    key = (gmax.tobytes(), ymax.tobytes())
    if key not in _NC_CACHE:
        _NC_CACHE[key] = _build_nc(gmax, ymax)
    nc = _NC_CACHE[key]

    def pshard(x, c):
        """[B, P, N] -> packed state layout [NT, 128, N], rows (p_l, b)."""
        sl = x[:, PL * c:PL * (c + 1), :]
        sl = np.transpose(sl, (1, 0, 2)).reshape(NT, 128, -1)
        return np.ascontiguousarray(sl)

    A0 = A[0]
    in_maps = []
    for c in range(NCORES):
        Asl = A0[PL * c:PL * (c + 1)].reshape(PL, 2, 128, N)
        # b vectors grouped 4 p's x 8 batches to match the col-tiled Atb build
        bsl = b[:, PL * c:PL * (c + 1), :, 0]            # [B, PL, M]
        bT = np.transpose(bsl, (2, 1, 0)).reshape(2, 128, NG, 4, B)
        bT = np.transpose(bT, (2, 0, 1, 3, 4)).reshape(NG, 2, 128, 32)

        # per-partition scalar columns
        scalp = np.zeros((128, 2 * K * NT), np.float32)
        for k in range(K):
            for t in range(NT):
                pg = PL * c + PPT * t + np.arange(128) // 8
                scalp[:, 2 * (k * NT + t)] = hyp[k, pg, 0]
                scalp[:, 2 * (k * NT + t) + 1] = hyp[k, pg, 1]
        scalb = np.zeros((128, 2 * (K - 1) * 2 + 4), np.float32)
        for kb in range(K - 1):
            for t in range(2):
                pg = 128 * t + np.arange(128)
                scalb[:, 2 * (kb * 2 + t)] = (hyp[kb + 1, pg, 2]
                                              + hyp[kb, pg, 3] * deg[c, pg])
                scalb[:, 2 * (kb * 2 + t) + 1] = hyp[kb, pg, 3]
        for t in range(2):
            scalb[:, 2 * (K - 1) * 2 + t] = 2.0 * deg[c, 128 * t:128 * (t + 1)]
            scalb[:, 2 * (K - 1) * 2 + 2 + t] = deg[c, 128 * t:128 * (t + 1)]

        in_maps.append({
            "A_in": np.ascontiguousarray(Asl),
            "bT_in": np.ascontiguousarray(bT),
            "y0_in": pshard(y0[..., 0], c),
            "c0_in": pshard(C0, c),
            "ub0_in": np.ascontiguousarray(U0[c, :, :, 0].reshape(2, 128, N)),
            "adj_in": np.ascontiguousarray(Adj[c].reshape(2, 128, P)),
            "scalp_in": scalp,
            "scalb_in": scalb,
            "ident_in": np.eye(128, dtype=np.float32),
        })

    res = run_bass_kernel_spmd(nc, in_maps, list(range(NCORES)))

    Y = np.empty((K, B, P, N, 1), np.float32)
    for c in range(NCORES):
        Y[:, :, PL * c:PL * (c + 1), :, 0] = res.results[c]["Y_out"]
    hyp_last = hyp[-1][..., None]
    return Y, hyp_last
